# revision 3
# baseline (speedup 1.0000x reference)
"""DistMult edge scorer on 8 Trainium2 NeuronCores.

score[r, e] = sigmoid(sum_d h_u[src[r,e], d] * W[r, d] * h_v[dst[r,e], d])

Sharding: edges of each relation are sorted by source node on the host and
split into 8 contiguous slices (one per core).  Each core receives:
  - the full destination tables (gathered per edge via SWDGE dma_gather)
  - only its source-range rows (prescaled by W[r] on device, staged to DRAM
    scratch, then gathered per edge)
Per 3584-edge batch: two dma_gathers (512B rows -> [128, 28, 128] tiles,
edges on partitions), one DVE multiply, one DVE 3D reduce; sigmoid on ACT
once per relation; scores DMA'd out contiguously and unpermuted on host.
"""

import numpy as np

N_DRUG, N_DIS, D = 8000, 18000, 128
N_REL_DIR, E = 3, 200000
N_CORES = 8
EPC = E // N_CORES          # 25000 edges per core per relation
EL = 25088                  # padded to 196 * 128
T = EL // 128               # 196 chunks per relation per core
NB = 3584                   # edges per gather batch (28 chunks)
NBATCH = EL // NB           # 7
BCH = NB // 128             # 28 chunks per batch
BIW = NB // 16              # 224 idx columns per batch

_cache = {}
_last = {}


def _build_nc(r_fwd: int, r_rev: int):
    import concourse.bacc as bacc
    import concourse.mybir as mybir
    from concourse.tile import TileContext

    f32 = mybir.dt.float32
    i16 = mybir.dt.int16

    nc = bacc.Bacc("TRN2", target_bir_lowering=False, debug=False,
                   num_devices=N_CORES)

    t_hd = nc.dram_tensor("hd", (N_DRUG, D), f32, kind="ExternalInput")
    t_hs = nc.dram_tensor("hs", (N_DIS, D), f32, kind="ExternalInput")
    t_uf = [nc.dram_tensor(f"uf{r}", (r_fwd, D), f32, kind="ExternalInput")
            for r in range(3)]
    t_ur = [nc.dram_tensor(f"ur{r}", (r_rev, D), f32, kind="ExternalInput")
            for r in range(3)]
    t_wb = nc.dram_tensor("wb", (128, 6, D), f32, kind="ExternalInput")
    t_iu = nc.dram_tensor("iu", (6, 128, EL // 16), i16, kind="ExternalInput")
    t_iv = nc.dram_tensor("iv", (6, 128, EL // 16), i16, kind="ExternalInput")
    t_out = nc.dram_tensor("scores", (6, 128, T), f32, kind="ExternalOutput")
    t_sf = [nc.dram_tensor(f"sf{r}", (r_fwd, D), f32, kind="Internal")
            for r in range(3)]
    t_sr = [nc.dram_tensor(f"sr{r}", (r_rev, D), f32, kind="Internal")
            for r in range(3)]

    with TileContext(nc) as tc:
        # Phase A: prescale the per-core u-range tables by W[r] -> DRAM scratch.
        with tc.tile_pool(name="pre", bufs=3) as pre:
            wb = pre.tile([128, 6, D], f32, tag="wb")
            nc.sync.dma_start(wb[:], t_wb[:])
            for r in range(6):
                src = t_uf[r] if r < 3 else t_ur[r - 3]
                dst = t_sf[r] if r < 3 else t_sr[r - 3]
                rows = r_fwd if r < 3 else r_rev
                nchunk = rows // 128
                src3 = src[:].rearrange("(c p) d -> p c d", p=128)
                dst3 = dst[:].rearrange("(c p) d -> p c d", p=128)
                for c0 in range(0, nchunk, 8):
                    cn = min(8, nchunk - c0)
                    stage = pre.tile([128, 8, D], f32, tag="stage")
                    nc.sync.dma_start(stage[:, :cn, :], src3[:, c0:c0 + cn, :])
                    for c in range(cn):
                        nc.vector.tensor_tensor(
                            stage[:, c, :], stage[:, c, :], wb[:, r, :],
                            op=mybir.AluOpType.mult)
                    nc.sync.dma_start(dst3[:, c0:c0 + cn, :], stage[:, :cn, :])

        # Phase B: per-edge gather + multiply + reduce + sigmoid.
        with tc.tile_pool(name="main", bufs=2) as mp, \
             tc.tile_pool(name="gp", bufs=3) as gp:
            for r in range(6):
                u_tab = t_sf[r] if r < 3 else t_sr[r - 3]
                v_tab = t_hs if r < 3 else t_hd
                iu = mp.tile([128, EL // 16], i16, tag="iu")
                iv = mp.tile([128, EL // 16], i16, tag="iv")
                nc.sync.dma_start(iu[:], t_iu[r])
                nc.sync.dma_start(iv[:], t_iv[r])
                scores = mp.tile([128, T], f32, tag="scores")
                for b in range(NBATCH):
                    gu = gp.tile([128, BCH, D], f32, tag="gu")
                    gv = gp.tile([128, BCH, D], f32, tag="gv")
                    nc.gpsimd.dma_gather(
                        gu[:], u_tab[:], iu[:, b * BIW:(b + 1) * BIW],
                        NB, NB, D, elem_step=D, single_packet=False)
                    nc.gpsimd.dma_gather(
                        gv[:], v_tab[:], iv[:, b * BIW:(b + 1) * BIW],
                        NB, NB, D, elem_step=D, single_packet=False)
                    prod = gp.tile([128, BCH, D], f32, tag="prod")
                    nc.vector.tensor_tensor(
                        prod[:].rearrange("p a b -> p (a b)"),
                        gu[:].rearrange("p a b -> p (a b)"),
                        gv[:].rearrange("p a b -> p (a b)"),
                        op=mybir.AluOpType.mult)
                    nc.vector.reduce_sum(
                        scores[:, b * BCH:(b + 1) * BCH], prod[:],
                        axis=mybir.AxisListType.X)
                sig = mp.tile([128, T], f32, tag="sig")
                nc.scalar.activation(
                    sig[:], scores[:], mybir.ActivationFunctionType.Sigmoid)
                nc.sync.dma_start(t_out[r], sig[:])

    nc.compile()
    return nc


def _wrap_idx(idx):
    # [EL] -> [128, EL//16]: position j lives at [16k + j%16, j//16] for all k.
    w = idx.reshape(EL // 16, 16).T.astype(np.int16)
    return np.ascontiguousarray(np.tile(w, (8, 1)))


def kernel(h_drug, h_disease, W, drug_src, dis_dst, dis_src, drug_dst):
    from concourse.bass_utils import run_bass_kernel_spmd

    h_drug = np.asarray(h_drug, dtype=np.float32)
    h_disease = np.asarray(h_disease, dtype=np.float32)
    W = np.asarray(W, dtype=np.float32)

    # relation -> (u index list, v index list, u table)
    rels = []
    for r in range(3):
        rels.append((np.asarray(drug_src[r]), np.asarray(dis_dst[r]), h_drug))
    for r in range(3):
        rels.append((np.asarray(dis_src[r]), np.asarray(drug_dst[r]), h_disease))

    perms = []           # per relation: argsort permutation
    core_slices = []     # per relation: list of (u_local, v_idx, lo, rows)
    for r in range(6):
        u_idx, v_idx, _ = rels[r]
        perm = np.argsort(u_idx, kind="stable")
        perms.append(perm)
        us, vs = u_idx[perm], v_idx[perm]
        sl = []
        for c in range(N_CORES):
            ui = us[c * EPC:(c + 1) * EPC]
            vi = vs[c * EPC:(c + 1) * EPC]
            pad = EL - EPC
            ui = np.concatenate([ui, np.full(pad, ui[-1], ui.dtype)])
            vi = np.concatenate([vi, np.full(pad, vi[-1], vi.dtype)])
            lo = int(ui.min())
            rows = int(ui.max()) - lo + 1
            sl.append((ui - lo, vi, lo, rows))
        core_slices.append(sl)

    def _pad128(n):
        return (n + 127) & ~127

    r_fwd = _pad128(max(core_slices[r][c][3] for r in range(3)
                        for c in range(N_CORES)))
    r_rev = _pad128(max(core_slices[r][c][3] for r in range(3, 6)
                        for c in range(N_CORES)))

    key = (r_fwd, r_rev)
    if key not in _cache:
        _cache[key] = _build_nc(r_fwd, r_rev)
    nc = _cache[key]

    wb = np.ascontiguousarray(np.broadcast_to(W[None, :, :], (128, 6, D)),
                              dtype=np.float32)

    def _rows(tab, lo, nrows):
        out = np.zeros((nrows, D), np.float32)
        n = min(nrows, tab.shape[0] - lo)
        out[:n] = tab[lo:lo + n]
        return out

    in_maps = []
    for c in range(N_CORES):
        m = {"hd": h_drug, "hs": h_disease, "wb": wb}
        iu = np.empty((6, 128, EL // 16), np.int16)
        iv = np.empty((6, 128, EL // 16), np.int16)
        for r in range(6):
            u_local, v_idx, lo, _ = core_slices[r][c]
            iu[r] = _wrap_idx(u_local.astype(np.int16))
            iv[r] = _wrap_idx(v_idx.astype(np.int16))
            tab = rels[r][2]
            nrows = r_fwd if r < 3 else r_rev
            name = f"uf{r}" if r < 3 else f"ur{r - 3}"
            m[name] = _rows(tab, lo, nrows)
        m["iu"] = iu
        m["iv"] = iv
        in_maps.append(m)

    res = run_bass_kernel_spmd(nc, in_maps, core_ids=list(range(N_CORES)))
    _last["exec_time_ns"] = res.exec_time_ns
    if res.instructions_and_trace is not None:
        _last["trace_path"] = res.instructions_and_trace[1]

    out = np.empty((6, E), np.float32)
    for r in range(6):
        parts = []
        for c in range(N_CORES):
            s = res.results[c]["scores"][r]          # [128, T]
            parts.append(s.T.reshape(-1)[:EPC])      # sorted-order scores
        sorted_scores = np.concatenate(parts)
        out[r, perms[r]] = sorted_scores
    return out


# revision 36
# speedup vs baseline: 3.0990x; 3.0990x over previous
"""DistMult edge scorer on 8 Trainium2 NeuronCores.

score[r, e] = sigmoid(sum_d h_u[src[r,e], d] * W[r, d] * h_v[dst[r,e], d])

Sharding: edges of each relation are sorted by source node on the host and
split into 8 contiguous slices (one per core).

Per core, per relation:
  - u side: the core's contiguous source-row range is DMA'd into SBUF once,
    prescaled by W[r] (DVE), and expanded per edge by PE one-hot selection
    matmuls.  Chunk t of 128 edges may only use source rows inside a
    two-block window [128*B_t, 128*(B_t+2)) where B_t = floor(t*NBLK/T2) is
    compile-time; the host packs edges greedily into chunks under that
    constraint (uniform data tracks the linear schedule closely).
  - v side: per-edge rows fetched with SWDGE dma_gather (512B rows,
    edges-on-partitions).  This is the bottleneck: the gather ucode costs
    ~8 ns per index on the Pool engine regardless of elem size.
  - DVE builds the one-hot masks (iota==ids) and does multiply+reduce;
    ACT applies sigmoid; scores are DMA'd out contiguously and unpermuted
    on the host.
"""

import numpy as np

N_DRUG, N_DIS, D = 8000, 18000, 128
N_REL_DIR, E = 3, 200000
N_CORES = 8
EPC = E // N_CORES          # 25000 edges per core per relation
T2 = 200                    # chunks per (relation, core); 25600 edge slots
EL = T2 * 128

_cache = {}
_last = {}


def _blk_of(t, nb):
    return min(t * (nb - 1) // T2, nb - 2)


def _build_nc(cfg):
    import concourse.bacc as bacc
    import concourse.mybir as mybir
    from concourse.tile import TileContext

    f32 = mybir.dt.float32
    i16 = mybir.dt.int16
    u8 = mybir.dt.uint8

    nblk_f, nblk_r, _t2 = cfg
    assert _t2 == T2
    nblk = {0: nblk_f, 1: nblk_r}

    nc = bacc.Bacc("TRN2", target_bir_lowering=False, debug=False,
                   num_devices=N_CORES, num_swdge_queues=4)

    t_hd = nc.dram_tensor("hd", (N_DRUG, D), f32, kind="ExternalInput")
    t_hs = nc.dram_tensor("hs", (N_DIS, D), f32, kind="ExternalInput")
    t_u = [nc.dram_tensor(f"u{r}", (nblk[r >= 3] * 128, D), f32,
                          kind="ExternalInput") for r in range(6)]
    t_wb = nc.dram_tensor("wb", (128, 6, D), f32, kind="ExternalInput")
    t_iota = nc.dram_tensor("iota", (128, 2), u8, kind="ExternalInput")
    t_ids = [nc.dram_tensor(f"ids{r}", (128, EL), u8,
                            kind="ExternalInput") for r in range(6)]
    t_iv = [nc.dram_tensor(f"iv{r}", (128, EL // 16), i16,
                           kind="ExternalInput") for r in range(6)]
    t_out = [nc.dram_tensor(f"scores{r}", (128, T2), f32,
                            kind="ExternalOutput") for r in range(6)]
    t_iu = [nc.dram_tensor(f"iu{r}", (128, EL // 16), i16,
                           kind="ExternalInput") for r in range(6)]
    t_us = [nc.dram_tensor(f"us{r}", (nblk[r >= 3] * 128, D), f32,
                           kind="Internal") for r in range(6)]

    with TileContext(nc) as tc:
        with tc.tile_pool(name="cst", bufs=1) as cst, \
             tc.tile_pool(name="mp", bufs=2) as mp, \
             tc.tile_pool(name="gp", bufs=2) as gp, \
             tc.tile_pool(name="pp", bufs=4, space="PSUM") as pp:
            wb = cst.tile([128, 6, D], f32)
            iota = cst.tile([128, 2], u8)
            nc.sync.dma_start(wb[:], t_wb[:])
            nc.sync.dma_start(iota[:], t_iota[:])
            for r in range(6):
                dr = int(r >= 3)
                NB = nblk[dr]
                v_tab = t_hs if dr == 0 else t_hd

                # u range -> SBUF (row 128b+p at [p, b, :]), prescale by W[r]
                u_sb = mp.tile([128, NB, D], f32, tag=f"usb{dr}")
                nc.sync.dma_start(
                    u_sb[:], t_u[r][:].rearrange("(b p) d -> p b d", p=128))
                for b in range(NB):
                    nc.vector.tensor_tensor(
                        u_sb[:, b, :], u_sb[:, b, :], wb[:, r, :],
                        op=mybir.AluOpType.mult)
                # scaled copy to DRAM scratch for the gathered-u chunks
                nc.sync.dma_start(
                    t_us[r][:].rearrange("(b p) d -> p b d", p=128), u_sb[:])

                iv = mp.tile([128, EL // 16], i16, tag="iv")
                nc.sync.dma_start(iv[:], t_iv[r][:])
                iu = mp.tile([128, EL // 16], i16, tag="iu")
                nc.sync.dma_start(iu[:], t_iu[r][:])
                scores = mp.tile([128, T2], f32, tag="scores")

                batches = [50] * (T2 // 50) + ([T2 % 50] if T2 % 50 else [])
                c0 = 0
                for b, kbn in enumerate(batches):
                    nb_i = kbn * 128
                    gv = gp.tile([128, 50, D], f32, tag="gv")
                    # split each batch across the 4 SWDGE queues: desc-gen for
                    # queue q runs on Q7 core pair q, so the four quarters
                    # generate concurrently
                    qn = kbn // 4
                    for q in range(4):
                        k0 = q * qn
                        k1 = kbn if q == 3 else (q + 1) * qn
                        nc.gpsimd.dma_gather(
                            gv[:, k0:k1, :], v_tab[:],
                            iv[:, (c0 + k0) * 8:(c0 + k1) * 8],
                            (k1 - k0) * 128, (k1 - k0) * 128, D,
                            elem_step=D, single_packet=False, queue_num=q)
                    # first gx chunks: u rows gathered from scaled DRAM
                    # scratch (Pool/SDMA path); rest: PE one-hot expansion
                    gx = min(8, ((2 * kbn) // 5) & ~3)
                    gu = gp.tile([128, 8, D], f32, tag="gu")
                    if gx > 0:
                        nc.gpsimd.dma_gather(
                            gu[:, :gx, :], t_us[r][:],
                            iu[:, c0 * 8:(c0 + gx) * 8],
                            gx * 128, gx * 128, D,
                            elem_step=D, single_packet=False, queue_num=b % 4)
                    noh = kbn - gx
                    ids = gp.tile([128, 42 * 128], u8, tag="ids")
                    nc.sync.dma_start(
                        ids[:, :noh * 128],
                        t_ids[r][:, (c0 + gx) * 128:(c0 + kbn) * 128])
                    oh_lo = gp.tile([128, 42 * 128], f32, tag="ohlo")
                    oh_hi = gp.tile([128, 42 * 128], f32, tag="ohhi")
                    nc.vector.tensor_tensor(
                        oh_lo[:, :noh * 128], ids[:, :noh * 128],
                        iota[:, 0:1].to_broadcast([128, noh * 128]),
                        op=mybir.AluOpType.is_equal)
                    nc.vector.tensor_tensor(
                        oh_hi[:, :noh * 128], ids[:, :noh * 128],
                        iota[:, 1:2].to_broadcast([128, noh * 128]),
                        op=mybir.AluOpType.is_equal)
                    for g0 in range(0, kbn, 4):
                        gn = min(4, kbn - g0)
                        if g0 + gn <= gx:
                            usrc = gu[:, g0:g0 + gn, :]
                        elif g0 >= gx:
                            ps = pp.tile([128, 4, D], f32, tag="ps")
                            for i in range(g0, g0 + gn):
                                t = c0 + i
                                blk = _blk_of(t, NB)
                                j = i - gx
                                nc.tensor.matmul(
                                    ps[:, i - g0, :],
                                    lhsT=oh_lo[:, j * 128:(j + 1) * 128],
                                    rhs=u_sb[:, blk, :],
                                    start=True, stop=False)
                                nc.tensor.matmul(
                                    ps[:, i - g0, :],
                                    lhsT=oh_hi[:, j * 128:(j + 1) * 128],
                                    rhs=u_sb[:, blk + 1, :],
                                    start=False, stop=True)
                            usrc = ps[:, :gn, :]
                        else:
                            raise AssertionError("gx must be multiple of 4")
                        prod = gp.tile([128, 4, D], f32, tag="prod")
                        nc.vector.tensor_tensor(
                            prod[:, :gn, :].rearrange("p a b -> p (a b)"),
                            usrc.rearrange("p a b -> p (a b)"),
                            gv[:, g0:g0 + gn, :].rearrange("p a b -> p (a b)"),
                            op=mybir.AluOpType.mult)
                        # reduction split between scalar engine (4x slower
                        # per chunk but otherwise idle) and DVE
                        if (g0 // 4) % 4 == 0:
                            nc.vector.reduce_sum(
                                scores[:, c0 + g0:c0 + g0 + gn],
                                prod[:, :gn, :], axis=mybir.AxisListType.X)
                        else:
                            acts = cst.tile([128, D], f32, tag="actout")
                            for i in range(gn):
                                nc.scalar.activation(
                                    acts[:], prod[:, i, :],
                                    mybir.ActivationFunctionType.Copy,
                                    accum_out=scores[:, c0 + g0 + i:c0 + g0 + i + 1])
                    c0 += kbn

                sig = mp.tile([128, T2], f32, tag="sig")
                nc.scalar.activation(
                    sig[:], scores[:], mybir.ActivationFunctionType.Sigmoid)
                nc.sync.dma_start(t_out[r][:], sig[:])

    nc.compile()
    return nc


def _wrap_idx(idx):
    n = idx.shape[0]
    w = idx.reshape(n // 16, 16).T.astype(np.int16)
    return np.ascontiguousarray(np.tile(w, (8, 1)))


def _pack_schedule(u_local, v_idx, nblk):
    """Greedy pack sorted edges into T2 chunks of 128 under the two-block
    window [128*B_t, 128*(B_t+2)).  Returns (ids_u8, v16, slot_of_edge)."""
    n = u_local.shape[0]
    ids = np.zeros(EL, np.uint8)
    v16 = np.zeros(EL, np.int16)
    edge_of_slot = np.full(EL, -1, np.int64)
    ptr = 0
    for t in range(T2):
        bt = min(t * (nblk - 1) // T2, nblk - 2)
        lo_row, hi_row = 128 * bt, 128 * (bt + 2)
        if ptr < n and u_local[ptr] < lo_row:
            raise RuntimeError("schedule fell behind data")
        # edges are sorted; find how many fit this window
        hi = np.searchsorted(u_local, hi_row, side="left")
        take = min(128, hi - ptr)
        if take > 0:
            s0 = t * 128
            ids[s0:s0 + take] = (u_local[ptr:ptr + take] - lo_row).astype(np.uint8)
            v16[s0:s0 + take] = v_idx[ptr:ptr + take].astype(np.int16)
            edge_of_slot[s0:s0 + take] = np.arange(ptr, ptr + take)
            # dummy slots replicate window base row with v=0 (harmless)
            ptr += take
    if ptr != n:
        raise RuntimeError(f"schedule failed to place all edges ({ptr}/{n})")
    return ids, v16, edge_of_slot


def _prepare(rels, sliced, nblk_f, nblk_r, wb, iota, h_drug, h_disease):
    slot_maps = [[None] * N_CORES for _ in range(6)]
    in_maps = []
    for c in range(N_CORES):
        m = {"hd": h_drug, "hs": h_disease, "wb": wb, "iota": iota}
        for r in range(6):
            dr = int(r >= 3)
            nblk = nblk_f if dr == 0 else nblk_r
            u_local, v_idx, lo = sliced[r][c]
            # Remap this core's rows to virtual rows spread by edge-count CDF
            # over [0, 128*(nblk-1)), so the data tracks the shared linear
            # chunk->block schedule exactly on every core.
            span = int(u_local[-1]) + 1
            V = 128 * (nblk - 1)
            counts = np.bincount(u_local, minlength=span).astype(np.int64)
            cum = np.concatenate([[0], np.cumsum(counts)[:-1]])
            target = (cum * V) // max(int(counts.sum()), 1)
            # strictly increasing: vpos[j] = max(target[j], vpos[j-1]+1)
            vpos = np.maximum.accumulate(target - np.arange(span)) + np.arange(span)
            if not vpos[-1] < nblk * 128:
                raise RuntimeError("virtual row remap overflow")
            u_virt = vpos[u_local]
            ids, v16, edge_of_slot = _pack_schedule(u_virt, v_idx, nblk)
            nrows = nblk * 128
            tab = rels[r][2]
            urows = np.zeros((nrows, D), np.float32)
            nn = min(span, tab.shape[0] - lo)
            urows[vpos[:nn]] = tab[lo:lo + nn]
            m[f"u{r}"] = urows
            m[f"ids{r}"] = np.ascontiguousarray(
                np.broadcast_to(ids[None, :], (128, EL)))
            m[f"iv{r}"] = _wrap_idx(v16)
            blk_arr = np.array([_blk_of(t, nblk) for t in range(T2)], np.int64)
            iu16 = (np.repeat(blk_arr, 128) * 128
                    + ids.astype(np.int64)).astype(np.int16)
            m[f"iu{r}"] = _wrap_idx(iu16)
            slot_maps[r][c] = edge_of_slot
        in_maps.append(m)
    return slot_maps, in_maps


def kernel(h_drug, h_disease, W, drug_src, dis_dst, dis_src, drug_dst):
    from concourse.bass_utils import run_bass_kernel_spmd

    h_drug = np.asarray(h_drug, dtype=np.float32)
    h_disease = np.asarray(h_disease, dtype=np.float32)
    W = np.asarray(W, dtype=np.float32)

    rels = []
    for r in range(3):
        rels.append((np.asarray(drug_src[r]), np.asarray(dis_dst[r]), h_drug))
    for r in range(3):
        rels.append((np.asarray(dis_src[r]), np.asarray(drug_dst[r]), h_disease))

    perms = []
    sliced = []
    for r in range(6):
        u_idx, v_idx, _ = rels[r]
        perm = np.argsort(u_idx, kind="stable")
        perms.append(perm)
        us, vs = u_idx[perm], v_idx[perm]
        sl = []
        for c in range(N_CORES):
            ui = us[c * EPC:(c + 1) * EPC]
            vi = vs[c * EPC:(c + 1) * EPC]
            lo = int(ui[0])
            sl.append((ui - lo, vi, lo))
        sliced.append(sl)

    def max_blocks(dr):
        nb = 2
        for r in (range(3) if dr == 0 else range(3, 6)):
            for c in range(N_CORES):
                u_local = sliced[r][c][0]
                nb = max(nb, int(u_local[-1]) // 128 + 2)
        return nb

    nblk_f, nblk_r = max_blocks(0), max_blocks(1)

    wb = np.ascontiguousarray(np.broadcast_to(W[None, :, :], (128, 6, D)),
                              dtype=np.float32)
    iota = np.empty((128, 2), np.uint8)
    iota[:, 0] = np.arange(128)
    iota[:, 1] = np.arange(128, 256)

    global T2, EL
    for _attempt in range(4):
        try:
            slot_maps, in_maps = _prepare(rels, sliced, nblk_f, nblk_r,
                                          wb, iota, h_drug, h_disease)
            break
        except RuntimeError:
            # pathological row distribution: give the schedule more slack
            T2 += 8
            EL = T2 * 128
    else:
        raise RuntimeError("could not build a feasible chunk schedule")

    cfg = (nblk_f, nblk_r, T2)
    if cfg not in _cache:
        _cache[cfg] = _build_nc(cfg)
    nc = _cache[cfg]

    res = run_bass_kernel_spmd(nc, in_maps, core_ids=list(range(N_CORES)))
    _last["exec_time_ns"] = res.exec_time_ns
    if res.instructions_and_trace is not None:
        _last["trace_path"] = res.instructions_and_trace[1]

    out = np.empty((6, E), np.float32)
    for r in range(6):
        sorted_scores = np.empty(EPC * N_CORES, np.float32)
        for c in range(N_CORES):
            s = res.results[c][f"scores{r}"]       # [128, T2]
            flat = s.T.reshape(-1)                 # slot j = t*128+p
            eos = slot_maps[r][c]
            valid = eos >= 0
            sorted_scores[c * EPC + eos[valid]] = flat[valid]
        out[r, perms[r]] = sorted_scores
    return out


# revision 37
# speedup vs baseline: 3.1366x; 1.0121x over previous
"""DistMult edge scorer on 8 Trainium2 NeuronCores.

score[r, e] = sigmoid(sum_d h_u[src[r,e], d] * W[r, d] * h_v[dst[r,e], d])

Sharding: edges of each relation are sorted by source node on the host and
split into 8 contiguous slices (one per core).

Per core, per relation:
  - u side: the core's contiguous source-row range is DMA'd into SBUF once,
    prescaled by W[r] (DVE), and expanded per edge by PE one-hot selection
    matmuls.  Chunk t of 128 edges may only use source rows inside a
    two-block window [128*B_t, 128*(B_t+2)) where B_t = floor(t*NBLK/T2) is
    compile-time; the host packs edges greedily into chunks under that
    constraint (uniform data tracks the linear schedule closely).
  - v side: per-edge rows fetched with SWDGE dma_gather (512B rows,
    edges-on-partitions).  This is the bottleneck: the gather ucode costs
    ~8 ns per index on the Pool engine regardless of elem size.
  - DVE builds the one-hot masks (iota==ids) and does multiply+reduce;
    ACT applies sigmoid; scores are DMA'd out contiguously and unpermuted
    on the host.
"""

import numpy as np

N_DRUG, N_DIS, D = 8000, 18000, 128
N_REL_DIR, E = 3, 200000
N_CORES = 8
EPC = E // N_CORES          # 25000 edges per core per relation
T2 = 200                    # chunks per (relation, core); 25600 edge slots
EL = T2 * 128

_cache = {}
_last = {}


def _blk_of(t, nb):
    return min(t * (nb - 1) // T2, nb - 2)


def _build_nc(cfg):
    import concourse.bacc as bacc
    import concourse.mybir as mybir
    from concourse.tile import TileContext

    f32 = mybir.dt.float32
    i16 = mybir.dt.int16
    u8 = mybir.dt.uint8

    nblk_f, nblk_r, _t2 = cfg
    assert _t2 == T2
    nblk = {0: nblk_f, 1: nblk_r}

    nc = bacc.Bacc("TRN2", target_bir_lowering=False, debug=False,
                   num_devices=N_CORES, num_swdge_queues=4)

    t_hd = nc.dram_tensor("hd", (N_DRUG, D), f32, kind="ExternalInput")
    t_hs = nc.dram_tensor("hs", (N_DIS, D), f32, kind="ExternalInput")
    t_u = [nc.dram_tensor(f"u{r}", (nblk[r >= 3] * 128, D), f32,
                          kind="ExternalInput") for r in range(6)]
    t_wb = nc.dram_tensor("wb", (128, 6, D), f32, kind="ExternalInput")
    t_iota = nc.dram_tensor("iota", (128, 2), u8, kind="ExternalInput")
    t_ids = [nc.dram_tensor(f"ids{r}", (128, EL), u8,
                            kind="ExternalInput") for r in range(6)]
    t_iv = [nc.dram_tensor(f"iv{r}", (128, EL // 16), i16,
                           kind="ExternalInput") for r in range(6)]
    t_out = [nc.dram_tensor(f"scores{r}", (128, T2), f32,
                            kind="ExternalOutput") for r in range(6)]
    t_iu = [nc.dram_tensor(f"iu{r}", (128, EL // 16), i16,
                           kind="ExternalInput") for r in range(6)]
    t_us = [nc.dram_tensor(f"us{r}", (nblk[r >= 3] * 128, D), f32,
                           kind="Internal") for r in range(6)]

    with TileContext(nc) as tc:
        with tc.tile_pool(name="cst", bufs=1) as cst, \
             tc.tile_pool(name="mp", bufs=2) as mp, \
             tc.tile_pool(name="gp", bufs=2) as gp, \
             tc.tile_pool(name="pp", bufs=4, space="PSUM") as pp:
            wb = cst.tile([128, 6, D], f32)
            iota = cst.tile([128, 2], u8)
            nc.sync.dma_start(wb[:], t_wb[:])
            nc.sync.dma_start(iota[:], t_iota[:])
            for r in range(6):
                dr = int(r >= 3)
                NB = nblk[dr]
                v_tab = t_hs if dr == 0 else t_hd

                # u range -> SBUF (row 128b+p at [p, b, :]), prescale by W[r]
                u_sb = mp.tile([128, NB, D], f32, tag=f"usb{dr}")
                nc.sync.dma_start(
                    u_sb[:], t_u[r][:].rearrange("(b p) d -> p b d", p=128))
                for b in range(NB):
                    nc.vector.tensor_tensor(
                        u_sb[:, b, :], u_sb[:, b, :], wb[:, r, :],
                        op=mybir.AluOpType.mult)
                # scaled copy to DRAM scratch for the gathered-u chunks
                nc.sync.dma_start(
                    t_us[r][:].rearrange("(b p) d -> p b d", p=128), u_sb[:])

                iv = mp.tile([128, EL // 16], i16, tag="iv")
                nc.sync.dma_start(iv[:], t_iv[r][:])
                iu = mp.tile([128, EL // 16], i16, tag="iu")
                nc.sync.dma_start(iu[:], t_iu[r][:])
                scores = mp.tile([128, T2], f32, tag="scores")

                batches = [50] * (T2 // 50) + ([T2 % 50] if T2 % 50 else [])
                c0 = 0
                for b, kbn in enumerate(batches):
                    nb_i = kbn * 128
                    gv = gp.tile([128, 50, D], f32, tag="gv")
                    # split each batch across the 4 SWDGE queues: desc-gen for
                    # queue q runs on Q7 core pair q, so the four quarters
                    # generate concurrently
                    # the queue that also carries this batch's u-gather gets
                    # a smaller v share so per-pair desc-gen is balanced
                    if kbn == 50:
                        sizes = [14, 14, 14, 14]
                        sizes[b % 4] = 8
                    else:
                        qn = kbn // 4
                        sizes = [qn, qn, qn, kbn - 3 * qn]
                    k0 = 0
                    for q in range(4):
                        k1 = k0 + sizes[q]
                        if k1 > k0:
                            nc.gpsimd.dma_gather(
                                gv[:, k0:k1, :], v_tab[:],
                                iv[:, (c0 + k0) * 8:(c0 + k1) * 8],
                                (k1 - k0) * 128, (k1 - k0) * 128, D,
                                elem_step=D, single_packet=False, queue_num=q)
                        k0 = k1
                    # first gx chunks: u rows gathered from scaled DRAM
                    # scratch (Pool/SDMA path); rest: PE one-hot expansion
                    gx = min(8, ((2 * kbn) // 5) & ~3)
                    gu = gp.tile([128, 8, D], f32, tag="gu")
                    if gx > 0:
                        nc.gpsimd.dma_gather(
                            gu[:, :gx, :], t_us[r][:],
                            iu[:, c0 * 8:(c0 + gx) * 8],
                            gx * 128, gx * 128, D,
                            elem_step=D, single_packet=False, queue_num=b % 4)
                    noh = kbn - gx
                    ids = gp.tile([128, 42 * 128], u8, tag="ids")
                    nc.sync.dma_start(
                        ids[:, :noh * 128],
                        t_ids[r][:, (c0 + gx) * 128:(c0 + kbn) * 128])
                    oh_lo = gp.tile([128, 42 * 128], f32, tag="ohlo")
                    oh_hi = gp.tile([128, 42 * 128], f32, tag="ohhi")
                    nc.vector.tensor_tensor(
                        oh_lo[:, :noh * 128], ids[:, :noh * 128],
                        iota[:, 0:1].to_broadcast([128, noh * 128]),
                        op=mybir.AluOpType.is_equal)
                    nc.vector.tensor_tensor(
                        oh_hi[:, :noh * 128], ids[:, :noh * 128],
                        iota[:, 1:2].to_broadcast([128, noh * 128]),
                        op=mybir.AluOpType.is_equal)
                    for g0 in range(0, kbn, 4):
                        gn = min(4, kbn - g0)
                        if g0 + gn <= gx:
                            usrc = gu[:, g0:g0 + gn, :]
                        elif g0 >= gx:
                            ps = pp.tile([128, 4, D], f32, tag="ps")
                            for i in range(g0, g0 + gn):
                                t = c0 + i
                                blk = _blk_of(t, NB)
                                j = i - gx
                                nc.tensor.matmul(
                                    ps[:, i - g0, :],
                                    lhsT=oh_lo[:, j * 128:(j + 1) * 128],
                                    rhs=u_sb[:, blk, :],
                                    start=True, stop=False)
                                nc.tensor.matmul(
                                    ps[:, i - g0, :],
                                    lhsT=oh_hi[:, j * 128:(j + 1) * 128],
                                    rhs=u_sb[:, blk + 1, :],
                                    start=False, stop=True)
                            usrc = ps[:, :gn, :]
                        else:
                            raise AssertionError("gx must be multiple of 4")
                        prod = gp.tile([128, 4, D], f32, tag="prod")
                        nc.vector.tensor_tensor(
                            prod[:, :gn, :].rearrange("p a b -> p (a b)"),
                            usrc.rearrange("p a b -> p (a b)"),
                            gv[:, g0:g0 + gn, :].rearrange("p a b -> p (a b)"),
                            op=mybir.AluOpType.mult)
                        # reduction split between scalar engine (4x slower
                        # per chunk but otherwise idle) and DVE
                        if (g0 // 4) % 4 == 0:
                            nc.vector.reduce_sum(
                                scores[:, c0 + g0:c0 + g0 + gn],
                                prod[:, :gn, :], axis=mybir.AxisListType.X)
                        else:
                            acts = cst.tile([128, D], f32, tag="actout")
                            for i in range(gn):
                                nc.scalar.activation(
                                    acts[:], prod[:, i, :],
                                    mybir.ActivationFunctionType.Copy,
                                    accum_out=scores[:, c0 + g0 + i:c0 + g0 + i + 1])
                    c0 += kbn

                sig = mp.tile([128, T2], f32, tag="sig")
                nc.scalar.activation(
                    sig[:], scores[:], mybir.ActivationFunctionType.Sigmoid)
                nc.sync.dma_start(t_out[r][:], sig[:])

    nc.compile()
    return nc


def _wrap_idx(idx):
    n = idx.shape[0]
    w = idx.reshape(n // 16, 16).T.astype(np.int16)
    return np.ascontiguousarray(np.tile(w, (8, 1)))


def _pack_schedule(u_local, v_idx, nblk):
    """Greedy pack sorted edges into T2 chunks of 128 under the two-block
    window [128*B_t, 128*(B_t+2)).  Returns (ids_u8, v16, slot_of_edge)."""
    n = u_local.shape[0]
    ids = np.zeros(EL, np.uint8)
    v16 = np.zeros(EL, np.int16)
    edge_of_slot = np.full(EL, -1, np.int64)
    ptr = 0
    for t in range(T2):
        bt = min(t * (nblk - 1) // T2, nblk - 2)
        lo_row, hi_row = 128 * bt, 128 * (bt + 2)
        if ptr < n and u_local[ptr] < lo_row:
            raise RuntimeError("schedule fell behind data")
        # edges are sorted; find how many fit this window
        hi = np.searchsorted(u_local, hi_row, side="left")
        take = min(128, hi - ptr)
        if take > 0:
            s0 = t * 128
            ids[s0:s0 + take] = (u_local[ptr:ptr + take] - lo_row).astype(np.uint8)
            v16[s0:s0 + take] = v_idx[ptr:ptr + take].astype(np.int16)
            edge_of_slot[s0:s0 + take] = np.arange(ptr, ptr + take)
            # dummy slots replicate window base row with v=0 (harmless)
            ptr += take
    if ptr != n:
        raise RuntimeError(f"schedule failed to place all edges ({ptr}/{n})")
    return ids, v16, edge_of_slot


def _prepare(rels, sliced, nblk_f, nblk_r, wb, iota, h_drug, h_disease):
    slot_maps = [[None] * N_CORES for _ in range(6)]
    in_maps = []
    for c in range(N_CORES):
        m = {"hd": h_drug, "hs": h_disease, "wb": wb, "iota": iota}
        for r in range(6):
            dr = int(r >= 3)
            nblk = nblk_f if dr == 0 else nblk_r
            u_local, v_idx, lo = sliced[r][c]
            # Remap this core's rows to virtual rows spread by edge-count CDF
            # over [0, 128*(nblk-1)), so the data tracks the shared linear
            # chunk->block schedule exactly on every core.
            span = int(u_local[-1]) + 1
            V = 128 * (nblk - 1)
            counts = np.bincount(u_local, minlength=span).astype(np.int64)
            cum = np.concatenate([[0], np.cumsum(counts)[:-1]])
            target = (cum * V) // max(int(counts.sum()), 1)
            # strictly increasing: vpos[j] = max(target[j], vpos[j-1]+1)
            vpos = np.maximum.accumulate(target - np.arange(span)) + np.arange(span)
            if not vpos[-1] < nblk * 128:
                raise RuntimeError("virtual row remap overflow")
            u_virt = vpos[u_local]
            ids, v16, edge_of_slot = _pack_schedule(u_virt, v_idx, nblk)
            nrows = nblk * 128
            tab = rels[r][2]
            urows = np.zeros((nrows, D), np.float32)
            nn = min(span, tab.shape[0] - lo)
            urows[vpos[:nn]] = tab[lo:lo + nn]
            m[f"u{r}"] = urows
            m[f"ids{r}"] = np.ascontiguousarray(
                np.broadcast_to(ids[None, :], (128, EL)))
            m[f"iv{r}"] = _wrap_idx(v16)
            blk_arr = np.array([_blk_of(t, nblk) for t in range(T2)], np.int64)
            iu16 = (np.repeat(blk_arr, 128) * 128
                    + ids.astype(np.int64)).astype(np.int16)
            m[f"iu{r}"] = _wrap_idx(iu16)
            slot_maps[r][c] = edge_of_slot
        in_maps.append(m)
    return slot_maps, in_maps


def kernel(h_drug, h_disease, W, drug_src, dis_dst, dis_src, drug_dst):
    from concourse.bass_utils import run_bass_kernel_spmd

    h_drug = np.asarray(h_drug, dtype=np.float32)
    h_disease = np.asarray(h_disease, dtype=np.float32)
    W = np.asarray(W, dtype=np.float32)

    rels = []
    for r in range(3):
        rels.append((np.asarray(drug_src[r]), np.asarray(dis_dst[r]), h_drug))
    for r in range(3):
        rels.append((np.asarray(dis_src[r]), np.asarray(drug_dst[r]), h_disease))

    perms = []
    sliced = []
    for r in range(6):
        u_idx, v_idx, _ = rels[r]
        perm = np.argsort(u_idx, kind="stable")
        perms.append(perm)
        us, vs = u_idx[perm], v_idx[perm]
        sl = []
        for c in range(N_CORES):
            ui = us[c * EPC:(c + 1) * EPC]
            vi = vs[c * EPC:(c + 1) * EPC]
            lo = int(ui[0])
            sl.append((ui - lo, vi, lo))
        sliced.append(sl)

    def max_blocks(dr):
        nb = 2
        for r in (range(3) if dr == 0 else range(3, 6)):
            for c in range(N_CORES):
                u_local = sliced[r][c][0]
                nb = max(nb, int(u_local[-1]) // 128 + 2)
        return nb

    nblk_f, nblk_r = max_blocks(0), max_blocks(1)

    wb = np.ascontiguousarray(np.broadcast_to(W[None, :, :], (128, 6, D)),
                              dtype=np.float32)
    iota = np.empty((128, 2), np.uint8)
    iota[:, 0] = np.arange(128)
    iota[:, 1] = np.arange(128, 256)

    global T2, EL
    for _attempt in range(4):
        try:
            slot_maps, in_maps = _prepare(rels, sliced, nblk_f, nblk_r,
                                          wb, iota, h_drug, h_disease)
            break
        except RuntimeError:
            # pathological row distribution: give the schedule more slack
            T2 += 8
            EL = T2 * 128
    else:
        raise RuntimeError("could not build a feasible chunk schedule")

    cfg = (nblk_f, nblk_r, T2)
    if cfg not in _cache:
        _cache[cfg] = _build_nc(cfg)
    nc = _cache[cfg]

    res = run_bass_kernel_spmd(nc, in_maps, core_ids=list(range(N_CORES)))
    _last["exec_time_ns"] = res.exec_time_ns
    if res.instructions_and_trace is not None:
        _last["trace_path"] = res.instructions_and_trace[1]

    out = np.empty((6, E), np.float32)
    for r in range(6):
        sorted_scores = np.empty(EPC * N_CORES, np.float32)
        for c in range(N_CORES):
            s = res.results[c][f"scores{r}"]       # [128, T2]
            flat = s.T.reshape(-1)                 # slot j = t*128+p
            eos = slot_maps[r][c]
            valid = eos >= 0
            sorted_scores[c * EPC + eos[valid]] = flat[valid]
        out[r, perms[r]] = sorted_scores
    return out


# revision 38
# speedup vs baseline: 3.3576x; 1.0705x over previous
"""DistMult edge scorer on 8 Trainium2 NeuronCores.

score[r, e] = sigmoid(sum_d h_u[src[r,e], d] * W[r, d] * h_v[dst[r,e], d])

Sharding: edges of each relation are sorted by source node on the host and
split into 8 contiguous slices (one per core).

Per core, per relation:
  - u side: the core's contiguous source-row range is DMA'd into SBUF once,
    prescaled by W[r] (DVE), and expanded per edge by PE one-hot selection
    matmuls.  Chunk t of 128 edges may only use source rows inside a
    two-block window [128*B_t, 128*(B_t+2)) where B_t = floor(t*NBLK/T2) is
    compile-time; the host packs edges greedily into chunks under that
    constraint (uniform data tracks the linear schedule closely).
  - v side: per-edge rows fetched with SWDGE dma_gather (512B rows,
    edges-on-partitions).  This is the bottleneck: the gather ucode costs
    ~8 ns per index on the Pool engine regardless of elem size.
  - DVE builds the one-hot masks (iota==ids) and does multiply+reduce;
    ACT applies sigmoid; scores are DMA'd out contiguously and unpermuted
    on the host.
"""

import numpy as np

N_DRUG, N_DIS, D = 8000, 18000, 128
N_REL_DIR, E = 3, 200000
N_CORES = 8
EPC = E // N_CORES          # 25000 edges per core per relation
T2 = 200                    # chunks per (relation, core); 25600 edge slots
EL = T2 * 128

_cache = {}
_last = {}


def _blk_of(t, nb):
    return min(t * (nb - 1) // T2, nb - 2)


def _build_nc(cfg):
    import concourse.bacc as bacc
    import concourse.mybir as mybir
    from concourse.tile import TileContext

    f32 = mybir.dt.float32
    i16 = mybir.dt.int16
    u8 = mybir.dt.uint8

    nblk_f, nblk_r, _t2 = cfg
    assert _t2 == T2
    nblk = {0: nblk_f, 1: nblk_r}

    nc = bacc.Bacc("TRN2", target_bir_lowering=False, debug=False,
                   num_devices=N_CORES, num_swdge_queues=4)

    t_hd = nc.dram_tensor("hd", (N_DRUG, D), f32, kind="ExternalInput")
    t_hs = nc.dram_tensor("hs", (N_DIS, D), f32, kind="ExternalInput")
    t_u = [nc.dram_tensor(f"u{r}", (nblk[r >= 3] * 128, D), f32,
                          kind="ExternalInput") for r in range(6)]
    t_wb = nc.dram_tensor("wb", (128, 6, D), f32, kind="ExternalInput")
    t_iota = nc.dram_tensor("iota", (128, 2), u8, kind="ExternalInput")
    t_ids = [nc.dram_tensor(f"ids{r}", (128, EL), u8,
                            kind="ExternalInput") for r in range(6)]
    t_iv = [nc.dram_tensor(f"iv{r}", (128, EL // 16), i16,
                           kind="ExternalInput") for r in range(6)]
    t_out = [nc.dram_tensor(f"scores{r}", (128, T2), f32,
                            kind="ExternalOutput") for r in range(6)]
    t_iu = [nc.dram_tensor(f"iu{r}", (128, EL // 16), i16,
                           kind="ExternalInput") for r in range(6)]
    t_us = [nc.dram_tensor(f"us{r}", (nblk[r >= 3] * 128, D), f32,
                           kind="Internal") for r in range(6)]

    with TileContext(nc) as tc:
        with tc.tile_pool(name="cst", bufs=1) as cst, \
             tc.tile_pool(name="mp", bufs=2) as mp, \
             tc.tile_pool(name="gp", bufs=2) as gp, \
             tc.tile_pool(name="pp", bufs=4, space="PSUM") as pp:
            wb = cst.tile([128, 6, D], f32)
            iota = cst.tile([128, 2], u8)
            nc.sync.dma_start(wb[:], t_wb[:])
            nc.sync.dma_start(iota[:], t_iota[:])
            for r in range(6):
                dr = int(r >= 3)
                NB = nblk[dr]
                v_tab = t_hs if dr == 0 else t_hd

                # u range -> SBUF (row 128b+p at [p, b, :]), prescale by W[r]
                u_sb = mp.tile([128, NB, D], f32, tag=f"usb{dr}")
                nc.sync.dma_start(
                    u_sb[:], t_u[r][:].rearrange("(b p) d -> p b d", p=128))
                for b in range(NB):
                    nc.vector.tensor_tensor(
                        u_sb[:, b, :], u_sb[:, b, :], wb[:, r, :],
                        op=mybir.AluOpType.mult)
                # scaled copy to DRAM scratch for the gathered-u chunks
                nc.sync.dma_start(
                    t_us[r][:].rearrange("(b p) d -> p b d", p=128), u_sb[:])

                iv = mp.tile([128, EL // 16], i16, tag="iv")
                nc.sync.dma_start(iv[:], t_iv[r][:])
                iu = mp.tile([128, EL // 16], i16, tag="iu")
                nc.sync.dma_start(iu[:], t_iu[r][:])
                scores = mp.tile([128, T2], f32, tag="scores")

                batches = [50] * (T2 // 50) + ([T2 % 50] if T2 % 50 else [])
                c0 = 0
                for b, kbn in enumerate(batches):
                    nb_i = kbn * 128
                    gv = gp.tile([128, 50, D], f32, tag="gv")
                    # split each batch across the 4 SWDGE queues: desc-gen for
                    # queue q runs on Q7 core pair q, so the four quarters
                    # generate concurrently
                    # the queue that also carries this batch's u-gather gets
                    # a smaller v share so per-pair desc-gen is balanced
                    if kbn == 50:
                        sizes = [17, 17, 17, 17]
                        sizes[b % 4] = 8
                        sizes[(b + 1) % 4] = 8
                    else:
                        qn = kbn // 4
                        sizes = [qn, qn, qn, kbn - 3 * qn]
                    k0 = 0
                    for q in range(4):
                        k1 = k0 + sizes[q]
                        if k1 > k0:
                            nc.gpsimd.dma_gather(
                                gv[:, k0:k1, :], v_tab[:],
                                iv[:, (c0 + k0) * 8:(c0 + k1) * 8],
                                (k1 - k0) * 128, (k1 - k0) * 128, D,
                                elem_step=D, single_packet=False, queue_num=q)
                        k0 = k1
                    # first gx chunks: u rows gathered from scaled DRAM
                    # scratch (Pool/SDMA path); rest: PE one-hot expansion
                    gx = min(16, ((2 * kbn) // 5) & ~3)
                    gu = gp.tile([128, 16, D], f32, tag="gu")
                    gh = gx // 2
                    if gx > 0:
                        nc.gpsimd.dma_gather(
                            gu[:, :gh, :], t_us[r][:],
                            iu[:, c0 * 8:(c0 + gh) * 8],
                            gh * 128, gh * 128, D,
                            elem_step=D, single_packet=False, queue_num=b % 4)
                        nc.gpsimd.dma_gather(
                            gu[:, gh:gx, :], t_us[r][:],
                            iu[:, (c0 + gh) * 8:(c0 + gx) * 8],
                            (gx - gh) * 128, (gx - gh) * 128, D,
                            elem_step=D, single_packet=False,
                            queue_num=(b + 1) % 4)
                    noh = kbn - gx
                    ids = gp.tile([128, 34 * 128], u8, tag="ids")
                    nc.sync.dma_start(
                        ids[:, :noh * 128],
                        t_ids[r][:, (c0 + gx) * 128:(c0 + kbn) * 128])
                    oh_lo = gp.tile([128, 34 * 128], f32, tag="ohlo")
                    oh_hi = gp.tile([128, 34 * 128], f32, tag="ohhi")
                    nc.vector.tensor_tensor(
                        oh_lo[:, :noh * 128], ids[:, :noh * 128],
                        iota[:, 0:1].to_broadcast([128, noh * 128]),
                        op=mybir.AluOpType.is_equal)
                    nc.vector.tensor_tensor(
                        oh_hi[:, :noh * 128], ids[:, :noh * 128],
                        iota[:, 1:2].to_broadcast([128, noh * 128]),
                        op=mybir.AluOpType.is_equal)
                    for g0 in range(0, kbn, 4):
                        gn = min(4, kbn - g0)
                        if g0 + gn <= gx:
                            usrc = gu[:, g0:g0 + gn, :]
                        elif g0 >= gx:
                            ps = pp.tile([128, 4, D], f32, tag="ps")
                            for i in range(g0, g0 + gn):
                                t = c0 + i
                                blk = _blk_of(t, NB)
                                j = i - gx
                                nc.tensor.matmul(
                                    ps[:, i - g0, :],
                                    lhsT=oh_lo[:, j * 128:(j + 1) * 128],
                                    rhs=u_sb[:, blk, :],
                                    start=True, stop=False)
                                nc.tensor.matmul(
                                    ps[:, i - g0, :],
                                    lhsT=oh_hi[:, j * 128:(j + 1) * 128],
                                    rhs=u_sb[:, blk + 1, :],
                                    start=False, stop=True)
                            usrc = ps[:, :gn, :]
                        else:
                            raise AssertionError("gx must be multiple of 4")
                        prod = gp.tile([128, 4, D], f32, tag="prod")
                        nc.vector.tensor_tensor(
                            prod[:, :gn, :].rearrange("p a b -> p (a b)"),
                            usrc.rearrange("p a b -> p (a b)"),
                            gv[:, g0:g0 + gn, :].rearrange("p a b -> p (a b)"),
                            op=mybir.AluOpType.mult)
                        # reduction split between scalar engine (4x slower
                        # per chunk but otherwise idle) and DVE
                        if (g0 // 4) % 3 == 0:
                            nc.vector.reduce_sum(
                                scores[:, c0 + g0:c0 + g0 + gn],
                                prod[:, :gn, :], axis=mybir.AxisListType.X)
                        else:
                            acts = cst.tile([128, D], f32, tag="actout")
                            for i in range(gn):
                                nc.scalar.activation(
                                    acts[:], prod[:, i, :],
                                    mybir.ActivationFunctionType.Copy,
                                    accum_out=scores[:, c0 + g0 + i:c0 + g0 + i + 1])
                    c0 += kbn

                sig = mp.tile([128, T2], f32, tag="sig")
                nc.scalar.activation(
                    sig[:], scores[:], mybir.ActivationFunctionType.Sigmoid)
                nc.sync.dma_start(t_out[r][:], sig[:])

    nc.compile()
    return nc


def _wrap_idx(idx):
    n = idx.shape[0]
    w = idx.reshape(n // 16, 16).T.astype(np.int16)
    return np.ascontiguousarray(np.tile(w, (8, 1)))


def _pack_schedule(u_local, v_idx, nblk):
    """Greedy pack sorted edges into T2 chunks of 128 under the two-block
    window [128*B_t, 128*(B_t+2)).  Returns (ids_u8, v16, slot_of_edge)."""
    n = u_local.shape[0]
    ids = np.zeros(EL, np.uint8)
    v16 = np.zeros(EL, np.int16)
    edge_of_slot = np.full(EL, -1, np.int64)
    ptr = 0
    for t in range(T2):
        bt = min(t * (nblk - 1) // T2, nblk - 2)
        lo_row, hi_row = 128 * bt, 128 * (bt + 2)
        if ptr < n and u_local[ptr] < lo_row:
            raise RuntimeError("schedule fell behind data")
        # edges are sorted; find how many fit this window
        hi = np.searchsorted(u_local, hi_row, side="left")
        take = min(128, hi - ptr)
        if take > 0:
            s0 = t * 128
            ids[s0:s0 + take] = (u_local[ptr:ptr + take] - lo_row).astype(np.uint8)
            v16[s0:s0 + take] = v_idx[ptr:ptr + take].astype(np.int16)
            edge_of_slot[s0:s0 + take] = np.arange(ptr, ptr + take)
            # dummy slots replicate window base row with v=0 (harmless)
            ptr += take
    if ptr != n:
        raise RuntimeError(f"schedule failed to place all edges ({ptr}/{n})")
    return ids, v16, edge_of_slot


def _prepare(rels, sliced, nblk_f, nblk_r, wb, iota, h_drug, h_disease):
    slot_maps = [[None] * N_CORES for _ in range(6)]
    in_maps = []
    for c in range(N_CORES):
        m = {"hd": h_drug, "hs": h_disease, "wb": wb, "iota": iota}
        for r in range(6):
            dr = int(r >= 3)
            nblk = nblk_f if dr == 0 else nblk_r
            u_local, v_idx, lo = sliced[r][c]
            # Remap this core's rows to virtual rows spread by edge-count CDF
            # over [0, 128*(nblk-1)), so the data tracks the shared linear
            # chunk->block schedule exactly on every core.
            span = int(u_local[-1]) + 1
            V = 128 * (nblk - 1)
            counts = np.bincount(u_local, minlength=span).astype(np.int64)
            cum = np.concatenate([[0], np.cumsum(counts)[:-1]])
            target = (cum * V) // max(int(counts.sum()), 1)
            # strictly increasing: vpos[j] = max(target[j], vpos[j-1]+1)
            vpos = np.maximum.accumulate(target - np.arange(span)) + np.arange(span)
            if not vpos[-1] < nblk * 128:
                raise RuntimeError("virtual row remap overflow")
            u_virt = vpos[u_local]
            ids, v16, edge_of_slot = _pack_schedule(u_virt, v_idx, nblk)
            nrows = nblk * 128
            tab = rels[r][2]
            urows = np.zeros((nrows, D), np.float32)
            nn = min(span, tab.shape[0] - lo)
            urows[vpos[:nn]] = tab[lo:lo + nn]
            m[f"u{r}"] = urows
            m[f"ids{r}"] = np.ascontiguousarray(
                np.broadcast_to(ids[None, :], (128, EL)))
            m[f"iv{r}"] = _wrap_idx(v16)
            blk_arr = np.array([_blk_of(t, nblk) for t in range(T2)], np.int64)
            iu16 = (np.repeat(blk_arr, 128) * 128
                    + ids.astype(np.int64)).astype(np.int16)
            m[f"iu{r}"] = _wrap_idx(iu16)
            slot_maps[r][c] = edge_of_slot
        in_maps.append(m)
    return slot_maps, in_maps


def kernel(h_drug, h_disease, W, drug_src, dis_dst, dis_src, drug_dst):
    from concourse.bass_utils import run_bass_kernel_spmd

    h_drug = np.asarray(h_drug, dtype=np.float32)
    h_disease = np.asarray(h_disease, dtype=np.float32)
    W = np.asarray(W, dtype=np.float32)

    rels = []
    for r in range(3):
        rels.append((np.asarray(drug_src[r]), np.asarray(dis_dst[r]), h_drug))
    for r in range(3):
        rels.append((np.asarray(dis_src[r]), np.asarray(drug_dst[r]), h_disease))

    perms = []
    sliced = []
    for r in range(6):
        u_idx, v_idx, _ = rels[r]
        perm = np.argsort(u_idx, kind="stable")
        perms.append(perm)
        us, vs = u_idx[perm], v_idx[perm]
        sl = []
        for c in range(N_CORES):
            ui = us[c * EPC:(c + 1) * EPC]
            vi = vs[c * EPC:(c + 1) * EPC]
            lo = int(ui[0])
            sl.append((ui - lo, vi, lo))
        sliced.append(sl)

    def max_blocks(dr):
        nb = 2
        for r in (range(3) if dr == 0 else range(3, 6)):
            for c in range(N_CORES):
                u_local = sliced[r][c][0]
                nb = max(nb, int(u_local[-1]) // 128 + 2)
        return nb

    nblk_f, nblk_r = max_blocks(0), max_blocks(1)

    wb = np.ascontiguousarray(np.broadcast_to(W[None, :, :], (128, 6, D)),
                              dtype=np.float32)
    iota = np.empty((128, 2), np.uint8)
    iota[:, 0] = np.arange(128)
    iota[:, 1] = np.arange(128, 256)

    global T2, EL
    for _attempt in range(4):
        try:
            slot_maps, in_maps = _prepare(rels, sliced, nblk_f, nblk_r,
                                          wb, iota, h_drug, h_disease)
            break
        except RuntimeError:
            # pathological row distribution: give the schedule more slack
            T2 += 8
            EL = T2 * 128
    else:
        raise RuntimeError("could not build a feasible chunk schedule")

    cfg = (nblk_f, nblk_r, T2)
    if cfg not in _cache:
        _cache[cfg] = _build_nc(cfg)
    nc = _cache[cfg]

    res = run_bass_kernel_spmd(nc, in_maps, core_ids=list(range(N_CORES)))
    _last["exec_time_ns"] = res.exec_time_ns
    if res.instructions_and_trace is not None:
        _last["trace_path"] = res.instructions_and_trace[1]

    out = np.empty((6, E), np.float32)
    for r in range(6):
        sorted_scores = np.empty(EPC * N_CORES, np.float32)
        for c in range(N_CORES):
            s = res.results[c][f"scores{r}"]       # [128, T2]
            flat = s.T.reshape(-1)                 # slot j = t*128+p
            eos = slot_maps[r][c]
            valid = eos >= 0
            sorted_scores[c * EPC + eos[valid]] = flat[valid]
        out[r, perms[r]] = sorted_scores
    return out


# revision 40
# speedup vs baseline: 3.6564x; 1.0890x over previous
"""DistMult edge scorer on 8 Trainium2 NeuronCores.

score[r, e] = sigmoid(sum_d h_u[src[r,e], d] * W[r, d] * h_v[dst[r,e], d])

Sharding: edges of each relation are sorted by source node on the host and
split into 8 contiguous slices (one per core).

Per core, per relation:
  - u side: the core's contiguous source-row range is DMA'd into SBUF once,
    prescaled by W[r] (DVE), and expanded per edge by PE one-hot selection
    matmuls.  Chunk t of 128 edges may only use source rows inside a
    two-block window [128*B_t, 128*(B_t+2)) where B_t = floor(t*NBLK/T2) is
    compile-time; the host packs edges greedily into chunks under that
    constraint (uniform data tracks the linear schedule closely).
  - v side: per-edge rows fetched with SWDGE dma_gather (512B rows,
    edges-on-partitions).  This is the bottleneck: the gather ucode costs
    ~8 ns per index on the Pool engine regardless of elem size.
  - DVE builds the one-hot masks (iota==ids) and does multiply+reduce;
    ACT applies sigmoid; scores are DMA'd out contiguously and unpermuted
    on the host.
"""

import numpy as np

N_DRUG, N_DIS, D = 8000, 18000, 128
N_REL_DIR, E = 3, 200000
N_CORES = 8
EPC = E // N_CORES          # 25000 edges per core per relation
T2 = 200                    # chunks per (relation, core); 25600 edge slots
EL = T2 * 128

_cache = {}
_last = {}


def _blk_of(t, nb):
    return min(t * (nb - 1) // T2, nb - 2)


def _build_nc(cfg):
    import concourse.bacc as bacc
    import concourse.mybir as mybir
    from concourse.tile import TileContext

    f32 = mybir.dt.float32
    i16 = mybir.dt.int16
    u8 = mybir.dt.uint8

    nblk_f, nblk_r, _t2 = cfg
    assert _t2 == T2
    nblk = {0: nblk_f, 1: nblk_r}

    nc = bacc.Bacc("TRN2", target_bir_lowering=False, debug=False,
                   num_devices=N_CORES, num_swdge_queues=4)

    t_hd = nc.dram_tensor("hd", (N_DRUG, D), f32, kind="ExternalInput")
    t_hs = nc.dram_tensor("hs", (N_DIS, D), f32, kind="ExternalInput")
    t_u = [nc.dram_tensor(f"u{r}", (nblk[r >= 3] * 128, D), f32,
                          kind="ExternalInput") for r in range(6)]
    t_wb = nc.dram_tensor("wb", (128, 6, D), f32, kind="ExternalInput")
    t_iota = nc.dram_tensor("iota", (128, 2), u8, kind="ExternalInput")
    t_ids = [nc.dram_tensor(f"ids{r}", (128, EL), u8,
                            kind="ExternalInput") for r in range(6)]
    t_iv = [nc.dram_tensor(f"iv{r}", (128, EL // 16), i16,
                           kind="ExternalInput") for r in range(6)]
    t_out = [nc.dram_tensor(f"scores{r}", (128, T2), f32,
                            kind="ExternalOutput") for r in range(6)]
    t_iu = [nc.dram_tensor(f"iu{r}", (128, EL // 16), i16,
                           kind="ExternalInput") for r in range(6)]
    t_us = [nc.dram_tensor(f"us{r}", (nblk[r >= 3] * 128, D), f32,
                           kind="Internal") for r in range(6)]

    with TileContext(nc) as tc:
        with tc.tile_pool(name="cst", bufs=1) as cst, \
             tc.tile_pool(name="mp", bufs=2) as mp, \
             tc.tile_pool(name="gp", bufs=2) as gp, \
             tc.tile_pool(name="gvp", bufs=3) as gvp, \
             tc.tile_pool(name="pp", bufs=4, space="PSUM") as pp:
            wb = cst.tile([128, 6, D], f32)
            iota = cst.tile([128, 2], u8)
            nc.sync.dma_start(wb[:], t_wb[:])
            nc.sync.dma_start(iota[:], t_iota[:])
            for r in range(6):
                dr = int(r >= 3)
                NB = nblk[dr]
                v_tab = t_hs if dr == 0 else t_hd

                # u range -> SBUF (row 128b+p at [p, b, :]), prescale by W[r]
                u_sb = mp.tile([128, NB, D], f32, tag=f"usb{dr}")
                nc.sync.dma_start(
                    u_sb[:], t_u[r][:].rearrange("(b p) d -> p b d", p=128))
                for b in range(NB):
                    nc.vector.tensor_tensor(
                        u_sb[:, b, :], u_sb[:, b, :], wb[:, r, :],
                        op=mybir.AluOpType.mult)
                # scaled copy to DRAM scratch for the gathered-u chunks
                nc.sync.dma_start(
                    t_us[r][:].rearrange("(b p) d -> p b d", p=128), u_sb[:])

                iv = mp.tile([128, EL // 16], i16, tag="iv")
                nc.sync.dma_start(iv[:], t_iv[r][:])
                iu = mp.tile([128, EL // 16], i16, tag="iu")
                nc.sync.dma_start(iu[:], t_iu[r][:])
                scores = mp.tile([128, T2], f32, tag="scores")

                batches = [40] * (T2 // 40) + ([T2 % 40] if T2 % 40 else [])
                c0 = 0
                for b, kbn in enumerate(batches):
                    nb_i = kbn * 128
                    gv = gvp.tile([128, 40, D], f32, tag="gv")
                    # split each batch across the 4 SWDGE queues: desc-gen for
                    # queue q runs on Q7 core pair q, so the four quarters
                    # generate concurrently
                    # the queue that also carries this batch's u-gather gets
                    # a smaller v share so per-pair desc-gen is balanced
                    if kbn == 40:
                        sizes = [14, 14, 14, 14]
                        sizes[b % 4] = 6
                        sizes[(b + 1) % 4] = 6
                    else:
                        qn = kbn // 4
                        sizes = [qn, qn, qn, kbn - 3 * qn]
                    k0 = 0
                    for q in range(4):
                        k1 = k0 + sizes[q]
                        if k1 > k0:
                            nc.gpsimd.dma_gather(
                                gv[:, k0:k1, :], v_tab[:],
                                iv[:, (c0 + k0) * 8:(c0 + k1) * 8],
                                (k1 - k0) * 128, (k1 - k0) * 128, D,
                                elem_step=D, single_packet=False, queue_num=q)
                        k0 = k1
                    # first gx chunks: u rows gathered from scaled DRAM
                    # scratch (Pool/SDMA path); rest: PE one-hot expansion
                    gx = min(16, ((2 * kbn) // 5) & ~3)
                    gu = gp.tile([128, 16, D], f32, tag="gu")
                    gh = gx // 2
                    if gx > 0:
                        nc.gpsimd.dma_gather(
                            gu[:, :gh, :], t_us[r][:],
                            iu[:, c0 * 8:(c0 + gh) * 8],
                            gh * 128, gh * 128, D,
                            elem_step=D, single_packet=False, queue_num=b % 4)
                        nc.gpsimd.dma_gather(
                            gu[:, gh:gx, :], t_us[r][:],
                            iu[:, (c0 + gh) * 8:(c0 + gx) * 8],
                            (gx - gh) * 128, (gx - gh) * 128, D,
                            elem_step=D, single_packet=False,
                            queue_num=(b + 1) % 4)
                    noh = kbn - gx
                    ids = gp.tile([128, 24 * 128], u8, tag="ids")
                    nc.sync.dma_start(
                        ids[:, :noh * 128],
                        t_ids[r][:, (c0 + gx) * 128:(c0 + kbn) * 128])
                    oh_lo = gp.tile([128, 24 * 128], f32, tag="ohlo")
                    oh_hi = gp.tile([128, 24 * 128], f32, tag="ohhi")
                    nc.vector.tensor_tensor(
                        oh_lo[:, :noh * 128], ids[:, :noh * 128],
                        iota[:, 0:1].to_broadcast([128, noh * 128]),
                        op=mybir.AluOpType.is_equal)
                    nc.vector.tensor_tensor(
                        oh_hi[:, :noh * 128], ids[:, :noh * 128],
                        iota[:, 1:2].to_broadcast([128, noh * 128]),
                        op=mybir.AluOpType.is_equal)
                    for g0 in range(0, kbn, 4):
                        gn = min(4, kbn - g0)
                        if g0 + gn <= gx:
                            usrc = gu[:, g0:g0 + gn, :]
                        elif g0 >= gx:
                            ps = pp.tile([128, 4, D], f32, tag="ps")
                            for i in range(g0, g0 + gn):
                                t = c0 + i
                                blk = _blk_of(t, NB)
                                j = i - gx
                                nc.tensor.matmul(
                                    ps[:, i - g0, :],
                                    lhsT=oh_lo[:, j * 128:(j + 1) * 128],
                                    rhs=u_sb[:, blk, :],
                                    start=True, stop=False)
                                nc.tensor.matmul(
                                    ps[:, i - g0, :],
                                    lhsT=oh_hi[:, j * 128:(j + 1) * 128],
                                    rhs=u_sb[:, blk + 1, :],
                                    start=False, stop=True)
                            usrc = ps[:, :gn, :]
                        else:
                            raise AssertionError("gx must be multiple of 4")
                        prod = gp.tile([128, 4, D], f32, tag="prod")
                        nc.vector.tensor_tensor(
                            prod[:, :gn, :].rearrange("p a b -> p (a b)"),
                            usrc.rearrange("p a b -> p (a b)"),
                            gv[:, g0:g0 + gn, :].rearrange("p a b -> p (a b)"),
                            op=mybir.AluOpType.mult)
                        # reduction split between scalar engine (4x slower
                        # per chunk but otherwise idle) and DVE
                        if (g0 // 4) % 3 == 0:
                            nc.vector.reduce_sum(
                                scores[:, c0 + g0:c0 + g0 + gn],
                                prod[:, :gn, :], axis=mybir.AxisListType.X)
                        else:
                            acts = cst.tile([128, D], f32, tag="actout")
                            for i in range(gn):
                                nc.scalar.activation(
                                    acts[:], prod[:, i, :],
                                    mybir.ActivationFunctionType.Copy,
                                    accum_out=scores[:, c0 + g0 + i:c0 + g0 + i + 1])
                    c0 += kbn

                sig = mp.tile([128, T2], f32, tag="sig")
                nc.scalar.activation(
                    sig[:], scores[:], mybir.ActivationFunctionType.Sigmoid)
                nc.sync.dma_start(t_out[r][:], sig[:])

    nc.compile()
    return nc


def _wrap_idx(idx):
    n = idx.shape[0]
    w = idx.reshape(n // 16, 16).T.astype(np.int16)
    return np.ascontiguousarray(np.tile(w, (8, 1)))


def _pack_schedule(u_local, v_idx, nblk):
    """Greedy pack sorted edges into T2 chunks of 128 under the two-block
    window [128*B_t, 128*(B_t+2)).  Returns (ids_u8, v16, slot_of_edge)."""
    n = u_local.shape[0]
    ids = np.zeros(EL, np.uint8)
    v16 = np.zeros(EL, np.int16)
    edge_of_slot = np.full(EL, -1, np.int64)
    ptr = 0
    for t in range(T2):
        bt = min(t * (nblk - 1) // T2, nblk - 2)
        lo_row, hi_row = 128 * bt, 128 * (bt + 2)
        if ptr < n and u_local[ptr] < lo_row:
            raise RuntimeError("schedule fell behind data")
        # edges are sorted; find how many fit this window
        hi = np.searchsorted(u_local, hi_row, side="left")
        take = min(128, hi - ptr)
        if take > 0:
            s0 = t * 128
            ids[s0:s0 + take] = (u_local[ptr:ptr + take] - lo_row).astype(np.uint8)
            v16[s0:s0 + take] = v_idx[ptr:ptr + take].astype(np.int16)
            edge_of_slot[s0:s0 + take] = np.arange(ptr, ptr + take)
            # dummy slots replicate window base row with v=0 (harmless)
            ptr += take
    if ptr != n:
        raise RuntimeError(f"schedule failed to place all edges ({ptr}/{n})")
    return ids, v16, edge_of_slot


def _prepare(rels, sliced, nblk_f, nblk_r, wb, iota, h_drug, h_disease):
    slot_maps = [[None] * N_CORES for _ in range(6)]
    in_maps = []
    for c in range(N_CORES):
        m = {"hd": h_drug, "hs": h_disease, "wb": wb, "iota": iota}
        for r in range(6):
            dr = int(r >= 3)
            nblk = nblk_f if dr == 0 else nblk_r
            u_local, v_idx, lo = sliced[r][c]
            # Remap this core's rows to virtual rows spread by edge-count CDF
            # over [0, 128*(nblk-1)), so the data tracks the shared linear
            # chunk->block schedule exactly on every core.
            span = int(u_local[-1]) + 1
            V = 128 * (nblk - 1)
            counts = np.bincount(u_local, minlength=span).astype(np.int64)
            cum = np.concatenate([[0], np.cumsum(counts)[:-1]])
            target = (cum * V) // max(int(counts.sum()), 1)
            # strictly increasing: vpos[j] = max(target[j], vpos[j-1]+1)
            vpos = np.maximum.accumulate(target - np.arange(span)) + np.arange(span)
            if not vpos[-1] < nblk * 128:
                raise RuntimeError("virtual row remap overflow")
            u_virt = vpos[u_local]
            ids, v16, edge_of_slot = _pack_schedule(u_virt, v_idx, nblk)
            nrows = nblk * 128
            tab = rels[r][2]
            urows = np.zeros((nrows, D), np.float32)
            nn = min(span, tab.shape[0] - lo)
            urows[vpos[:nn]] = tab[lo:lo + nn]
            m[f"u{r}"] = urows
            m[f"ids{r}"] = np.ascontiguousarray(
                np.broadcast_to(ids[None, :], (128, EL)))
            m[f"iv{r}"] = _wrap_idx(v16)
            blk_arr = np.array([_blk_of(t, nblk) for t in range(T2)], np.int64)
            iu16 = (np.repeat(blk_arr, 128) * 128
                    + ids.astype(np.int64)).astype(np.int16)
            m[f"iu{r}"] = _wrap_idx(iu16)
            slot_maps[r][c] = edge_of_slot
        in_maps.append(m)
    return slot_maps, in_maps


def kernel(h_drug, h_disease, W, drug_src, dis_dst, dis_src, drug_dst):
    from concourse.bass_utils import run_bass_kernel_spmd

    h_drug = np.asarray(h_drug, dtype=np.float32)
    h_disease = np.asarray(h_disease, dtype=np.float32)
    W = np.asarray(W, dtype=np.float32)

    rels = []
    for r in range(3):
        rels.append((np.asarray(drug_src[r]), np.asarray(dis_dst[r]), h_drug))
    for r in range(3):
        rels.append((np.asarray(dis_src[r]), np.asarray(drug_dst[r]), h_disease))

    perms = []
    sliced = []
    for r in range(6):
        u_idx, v_idx, _ = rels[r]
        perm = np.argsort(u_idx, kind="stable")
        perms.append(perm)
        us, vs = u_idx[perm], v_idx[perm]
        sl = []
        for c in range(N_CORES):
            ui = us[c * EPC:(c + 1) * EPC]
            vi = vs[c * EPC:(c + 1) * EPC]
            lo = int(ui[0])
            sl.append((ui - lo, vi, lo))
        sliced.append(sl)

    def max_blocks(dr):
        nb = 2
        for r in (range(3) if dr == 0 else range(3, 6)):
            for c in range(N_CORES):
                u_local = sliced[r][c][0]
                nb = max(nb, int(u_local[-1]) // 128 + 2)
        return nb

    nblk_f, nblk_r = max_blocks(0), max_blocks(1)

    wb = np.ascontiguousarray(np.broadcast_to(W[None, :, :], (128, 6, D)),
                              dtype=np.float32)
    iota = np.empty((128, 2), np.uint8)
    iota[:, 0] = np.arange(128)
    iota[:, 1] = np.arange(128, 256)

    global T2, EL
    for _attempt in range(4):
        try:
            slot_maps, in_maps = _prepare(rels, sliced, nblk_f, nblk_r,
                                          wb, iota, h_drug, h_disease)
            break
        except RuntimeError:
            # pathological row distribution: give the schedule more slack
            T2 += 8
            EL = T2 * 128
    else:
        raise RuntimeError("could not build a feasible chunk schedule")

    cfg = (nblk_f, nblk_r, T2)
    if cfg not in _cache:
        _cache[cfg] = _build_nc(cfg)
    nc = _cache[cfg]

    res = run_bass_kernel_spmd(nc, in_maps, core_ids=list(range(N_CORES)))
    _last["exec_time_ns"] = res.exec_time_ns
    if res.instructions_and_trace is not None:
        _last["trace_path"] = res.instructions_and_trace[1]

    out = np.empty((6, E), np.float32)
    for r in range(6):
        sorted_scores = np.empty(EPC * N_CORES, np.float32)
        for c in range(N_CORES):
            s = res.results[c][f"scores{r}"]       # [128, T2]
            flat = s.T.reshape(-1)                 # slot j = t*128+p
            eos = slot_maps[r][c]
            valid = eos >= 0
            sorted_scores[c * EPC + eos[valid]] = flat[valid]
        out[r, perms[r]] = sorted_scores
    return out


# revision 41
# speedup vs baseline: 3.7004x; 1.0120x over previous
"""DistMult edge scorer on 8 Trainium2 NeuronCores.

score[r, e] = sigmoid(sum_d h_u[src[r,e], d] * W[r, d] * h_v[dst[r,e], d])

Sharding: edges of each relation are sorted by source node on the host and
split into 8 contiguous slices (one per core).

Per core, per relation:
  - u side: the core's contiguous source-row range is DMA'd into SBUF once,
    prescaled by W[r] (DVE), and expanded per edge by PE one-hot selection
    matmuls.  Chunk t of 128 edges may only use source rows inside a
    two-block window [128*B_t, 128*(B_t+2)) where B_t = floor(t*NBLK/T2) is
    compile-time; the host packs edges greedily into chunks under that
    constraint (uniform data tracks the linear schedule closely).
  - v side: per-edge rows fetched with SWDGE dma_gather (512B rows,
    edges-on-partitions).  This is the bottleneck: the gather ucode costs
    ~8 ns per index on the Pool engine regardless of elem size.
  - DVE builds the one-hot masks (iota==ids) and does multiply+reduce;
    ACT applies sigmoid; scores are DMA'd out contiguously and unpermuted
    on the host.
"""

import numpy as np

N_DRUG, N_DIS, D = 8000, 18000, 128
N_REL_DIR, E = 3, 200000
N_CORES = 8
EPC = E // N_CORES          # 25000 edges per core per relation
T2 = 200                    # chunks per (relation, core); 25600 edge slots
EL = T2 * 128

_cache = {}
_last = {}


def _blk_of(t, nb):
    return min(t * (nb - 1) // T2, nb - 2)


def _build_nc(cfg):
    import concourse.bacc as bacc
    import concourse.mybir as mybir
    from concourse.tile import TileContext

    f32 = mybir.dt.float32
    i16 = mybir.dt.int16
    u8 = mybir.dt.uint8

    nblk_f, nblk_r, _t2 = cfg
    assert _t2 == T2
    nblk = {0: nblk_f, 1: nblk_r}

    nc = bacc.Bacc("TRN2", target_bir_lowering=False, debug=False,
                   num_devices=N_CORES, num_swdge_queues=4)

    t_hd = nc.dram_tensor("hd", (N_DRUG, D), f32, kind="ExternalInput")
    t_hs = nc.dram_tensor("hs", (N_DIS, D), f32, kind="ExternalInput")
    t_u = [nc.dram_tensor(f"u{r}", (nblk[r >= 3] * 128, D), f32,
                          kind="ExternalInput") for r in range(6)]
    t_wb = nc.dram_tensor("wb", (128, 6, D), f32, kind="ExternalInput")
    t_iota = nc.dram_tensor("iota", (128, 2), u8, kind="ExternalInput")
    t_ids = [nc.dram_tensor(f"ids{r}", (128, EL), u8,
                            kind="ExternalInput") for r in range(6)]
    t_iv = [nc.dram_tensor(f"iv{r}", (128, EL // 16), i16,
                           kind="ExternalInput") for r in range(6)]
    t_out = [nc.dram_tensor(f"scores{r}", (128, T2), f32,
                            kind="ExternalOutput") for r in range(6)]
    t_iu = [nc.dram_tensor(f"iu{r}", (128, EL // 16), i16,
                           kind="ExternalInput") for r in range(6)]
    t_us = [nc.dram_tensor(f"us{r}", (nblk[r >= 3] * 128, D), f32,
                           kind="Internal") for r in range(6)]

    with TileContext(nc) as tc:
        with tc.tile_pool(name="cst", bufs=1) as cst, \
             tc.tile_pool(name="mp", bufs=2) as mp, \
             tc.tile_pool(name="gp", bufs=2) as gp, \
             tc.tile_pool(name="gvp", bufs=3) as gvp, \
             tc.tile_pool(name="pp", bufs=4, space="PSUM") as pp:
            wb = cst.tile([128, 6, D], f32)
            iota = cst.tile([128, 2], u8)
            nc.sync.dma_start(wb[:], t_wb[:])
            nc.sync.dma_start(iota[:], t_iota[:])
            for r in range(6):
                dr = int(r >= 3)
                NB = nblk[dr]
                v_tab = t_hs if dr == 0 else t_hd

                # u range -> SBUF (row 128b+p at [p, b, :]), prescale by W[r]
                u_sb = mp.tile([128, NB, D], f32, tag=f"usb{dr}")
                nc.sync.dma_start(
                    u_sb[:], t_u[r][:].rearrange("(b p) d -> p b d", p=128))
                for b in range(NB):
                    nc.vector.tensor_tensor(
                        u_sb[:, b, :], u_sb[:, b, :], wb[:, r, :],
                        op=mybir.AluOpType.mult)
                # scaled copy to DRAM scratch for the gathered-u chunks
                nc.sync.dma_start(
                    t_us[r][:].rearrange("(b p) d -> p b d", p=128), u_sb[:])

                iv = mp.tile([128, EL // 16], i16, tag="iv")
                nc.sync.dma_start(iv[:], t_iv[r][:])
                iu = mp.tile([128, EL // 16], i16, tag="iu")
                nc.sync.dma_start(iu[:], t_iu[r][:])
                scores = mp.tile([128, T2], f32, tag="scores")

                batches = [40] * (T2 // 40) + ([T2 % 40] if T2 % 40 else [])
                c0 = 0
                for b, kbn in enumerate(batches):
                    nb_i = kbn * 128
                    gv = gvp.tile([128, 40, D], f32, tag="gv")
                    # split each batch across the 4 SWDGE queues: desc-gen for
                    # queue q runs on Q7 core pair q, so the four quarters
                    # generate concurrently
                    # the queue that also carries this batch's u-gather gets
                    # a smaller v share so per-pair desc-gen is balanced
                    # fine-grained, pair-balanced issue: pairs 0/1 take
                    # 2x7 v-chunks, pairs 2/3 take 6 v-chunks (they also
                    # carry the 8-chunk u-gathers) -> 14 chunks per pair
                    if kbn == 40:
                        segs = [(0, 7), (1, 7), (2, 6), (3, 6), (0, 7), (1, 7)]
                    else:
                        qn = max(1, kbn // 4)
                        segs = []
                        left, q = kbn, 0
                        while left > 0:
                            take = min(qn, left)
                            segs.append((q % 4, take))
                            left -= take
                            q += 1
                    k0 = 0
                    for q, sz in segs:
                        k1 = k0 + sz
                        nc.gpsimd.dma_gather(
                            gv[:, k0:k1, :], v_tab[:],
                            iv[:, (c0 + k0) * 8:(c0 + k1) * 8],
                            sz * 128, sz * 128, D,
                            elem_step=D, single_packet=False, queue_num=q)
                        k0 = k1
                    # first gx chunks: u rows gathered from scaled DRAM
                    # scratch (Pool/SDMA path); rest: PE one-hot expansion
                    gx = min(16, ((2 * kbn) // 5) & ~3)
                    gu = gp.tile([128, 16, D], f32, tag="gu")
                    gh = gx // 2
                    if gx > 0:
                        nc.gpsimd.dma_gather(
                            gu[:, :gh, :], t_us[r][:],
                            iu[:, c0 * 8:(c0 + gh) * 8],
                            gh * 128, gh * 128, D,
                            elem_step=D, single_packet=False, queue_num=2)
                        nc.gpsimd.dma_gather(
                            gu[:, gh:gx, :], t_us[r][:],
                            iu[:, (c0 + gh) * 8:(c0 + gx) * 8],
                            (gx - gh) * 128, (gx - gh) * 128, D,
                            elem_step=D, single_packet=False,
                            queue_num=3)
                    noh = kbn - gx
                    ids = gp.tile([128, 24 * 128], u8, tag="ids")
                    nc.sync.dma_start(
                        ids[:, :noh * 128],
                        t_ids[r][:, (c0 + gx) * 128:(c0 + kbn) * 128])
                    oh_lo = gp.tile([128, 24 * 128], f32, tag="ohlo")
                    oh_hi = gp.tile([128, 24 * 128], f32, tag="ohhi")
                    nc.vector.tensor_tensor(
                        oh_lo[:, :noh * 128], ids[:, :noh * 128],
                        iota[:, 0:1].to_broadcast([128, noh * 128]),
                        op=mybir.AluOpType.is_equal)
                    nc.vector.tensor_tensor(
                        oh_hi[:, :noh * 128], ids[:, :noh * 128],
                        iota[:, 1:2].to_broadcast([128, noh * 128]),
                        op=mybir.AluOpType.is_equal)
                    for g0 in range(0, kbn, 4):
                        gn = min(4, kbn - g0)
                        if g0 + gn <= gx:
                            usrc = gu[:, g0:g0 + gn, :]
                        elif g0 >= gx:
                            ps = pp.tile([128, 4, D], f32, tag="ps")
                            for i in range(g0, g0 + gn):
                                t = c0 + i
                                blk = _blk_of(t, NB)
                                j = i - gx
                                nc.tensor.matmul(
                                    ps[:, i - g0, :],
                                    lhsT=oh_lo[:, j * 128:(j + 1) * 128],
                                    rhs=u_sb[:, blk, :],
                                    start=True, stop=False)
                                nc.tensor.matmul(
                                    ps[:, i - g0, :],
                                    lhsT=oh_hi[:, j * 128:(j + 1) * 128],
                                    rhs=u_sb[:, blk + 1, :],
                                    start=False, stop=True)
                            usrc = ps[:, :gn, :]
                        else:
                            raise AssertionError("gx must be multiple of 4")
                        prod = gp.tile([128, 4, D], f32, tag="prod")
                        nc.vector.tensor_tensor(
                            prod[:, :gn, :].rearrange("p a b -> p (a b)"),
                            usrc.rearrange("p a b -> p (a b)"),
                            gv[:, g0:g0 + gn, :].rearrange("p a b -> p (a b)"),
                            op=mybir.AluOpType.mult)
                        # reduction split between scalar engine (4x slower
                        # per chunk but otherwise idle) and DVE
                        if (g0 // 4) % 3 == 0:
                            nc.vector.reduce_sum(
                                scores[:, c0 + g0:c0 + g0 + gn],
                                prod[:, :gn, :], axis=mybir.AxisListType.X)
                        else:
                            acts = cst.tile([128, D], f32, tag="actout")
                            for i in range(gn):
                                nc.scalar.activation(
                                    acts[:], prod[:, i, :],
                                    mybir.ActivationFunctionType.Copy,
                                    accum_out=scores[:, c0 + g0 + i:c0 + g0 + i + 1])
                    c0 += kbn

                sig = mp.tile([128, T2], f32, tag="sig")
                nc.scalar.activation(
                    sig[:], scores[:], mybir.ActivationFunctionType.Sigmoid)
                nc.sync.dma_start(t_out[r][:], sig[:])

    nc.compile()
    return nc


def _wrap_idx(idx):
    n = idx.shape[0]
    w = idx.reshape(n // 16, 16).T.astype(np.int16)
    return np.ascontiguousarray(np.tile(w, (8, 1)))


def _pack_schedule(u_local, v_idx, nblk):
    """Greedy pack sorted edges into T2 chunks of 128 under the two-block
    window [128*B_t, 128*(B_t+2)).  Returns (ids_u8, v16, slot_of_edge)."""
    n = u_local.shape[0]
    ids = np.zeros(EL, np.uint8)
    v16 = np.zeros(EL, np.int16)
    edge_of_slot = np.full(EL, -1, np.int64)
    ptr = 0
    for t in range(T2):
        bt = min(t * (nblk - 1) // T2, nblk - 2)
        lo_row, hi_row = 128 * bt, 128 * (bt + 2)
        if ptr < n and u_local[ptr] < lo_row:
            raise RuntimeError("schedule fell behind data")
        # edges are sorted; find how many fit this window
        hi = np.searchsorted(u_local, hi_row, side="left")
        take = min(128, hi - ptr)
        if take > 0:
            s0 = t * 128
            ids[s0:s0 + take] = (u_local[ptr:ptr + take] - lo_row).astype(np.uint8)
            v16[s0:s0 + take] = v_idx[ptr:ptr + take].astype(np.int16)
            edge_of_slot[s0:s0 + take] = np.arange(ptr, ptr + take)
            # dummy slots replicate window base row with v=0 (harmless)
            ptr += take
    if ptr != n:
        raise RuntimeError(f"schedule failed to place all edges ({ptr}/{n})")
    return ids, v16, edge_of_slot


def _prepare(rels, sliced, nblk_f, nblk_r, wb, iota, h_drug, h_disease):
    slot_maps = [[None] * N_CORES for _ in range(6)]
    in_maps = []
    for c in range(N_CORES):
        m = {"hd": h_drug, "hs": h_disease, "wb": wb, "iota": iota}
        for r in range(6):
            dr = int(r >= 3)
            nblk = nblk_f if dr == 0 else nblk_r
            u_local, v_idx, lo = sliced[r][c]
            # Remap this core's rows to virtual rows spread by edge-count CDF
            # over [0, 128*(nblk-1)), so the data tracks the shared linear
            # chunk->block schedule exactly on every core.
            span = int(u_local[-1]) + 1
            V = 128 * (nblk - 1)
            counts = np.bincount(u_local, minlength=span).astype(np.int64)
            cum = np.concatenate([[0], np.cumsum(counts)[:-1]])
            target = (cum * V) // max(int(counts.sum()), 1)
            # strictly increasing: vpos[j] = max(target[j], vpos[j-1]+1)
            vpos = np.maximum.accumulate(target - np.arange(span)) + np.arange(span)
            if not vpos[-1] < nblk * 128:
                raise RuntimeError("virtual row remap overflow")
            u_virt = vpos[u_local]
            ids, v16, edge_of_slot = _pack_schedule(u_virt, v_idx, nblk)
            nrows = nblk * 128
            tab = rels[r][2]
            urows = np.zeros((nrows, D), np.float32)
            nn = min(span, tab.shape[0] - lo)
            urows[vpos[:nn]] = tab[lo:lo + nn]
            m[f"u{r}"] = urows
            m[f"ids{r}"] = np.ascontiguousarray(
                np.broadcast_to(ids[None, :], (128, EL)))
            m[f"iv{r}"] = _wrap_idx(v16)
            blk_arr = np.array([_blk_of(t, nblk) for t in range(T2)], np.int64)
            iu16 = (np.repeat(blk_arr, 128) * 128
                    + ids.astype(np.int64)).astype(np.int16)
            m[f"iu{r}"] = _wrap_idx(iu16)
            slot_maps[r][c] = edge_of_slot
        in_maps.append(m)
    return slot_maps, in_maps


def kernel(h_drug, h_disease, W, drug_src, dis_dst, dis_src, drug_dst):
    from concourse.bass_utils import run_bass_kernel_spmd

    h_drug = np.asarray(h_drug, dtype=np.float32)
    h_disease = np.asarray(h_disease, dtype=np.float32)
    W = np.asarray(W, dtype=np.float32)

    rels = []
    for r in range(3):
        rels.append((np.asarray(drug_src[r]), np.asarray(dis_dst[r]), h_drug))
    for r in range(3):
        rels.append((np.asarray(dis_src[r]), np.asarray(drug_dst[r]), h_disease))

    perms = []
    sliced = []
    for r in range(6):
        u_idx, v_idx, _ = rels[r]
        perm = np.argsort(u_idx, kind="stable")
        perms.append(perm)
        us, vs = u_idx[perm], v_idx[perm]
        sl = []
        for c in range(N_CORES):
            ui = us[c * EPC:(c + 1) * EPC]
            vi = vs[c * EPC:(c + 1) * EPC]
            lo = int(ui[0])
            sl.append((ui - lo, vi, lo))
        sliced.append(sl)

    def max_blocks(dr):
        nb = 2
        for r in (range(3) if dr == 0 else range(3, 6)):
            for c in range(N_CORES):
                u_local = sliced[r][c][0]
                nb = max(nb, int(u_local[-1]) // 128 + 2)
        return nb

    nblk_f, nblk_r = max_blocks(0), max_blocks(1)

    wb = np.ascontiguousarray(np.broadcast_to(W[None, :, :], (128, 6, D)),
                              dtype=np.float32)
    iota = np.empty((128, 2), np.uint8)
    iota[:, 0] = np.arange(128)
    iota[:, 1] = np.arange(128, 256)

    global T2, EL
    for _attempt in range(4):
        try:
            slot_maps, in_maps = _prepare(rels, sliced, nblk_f, nblk_r,
                                          wb, iota, h_drug, h_disease)
            break
        except RuntimeError:
            # pathological row distribution: give the schedule more slack
            T2 += 8
            EL = T2 * 128
    else:
        raise RuntimeError("could not build a feasible chunk schedule")

    cfg = (nblk_f, nblk_r, T2)
    if cfg not in _cache:
        _cache[cfg] = _build_nc(cfg)
    nc = _cache[cfg]

    res = run_bass_kernel_spmd(nc, in_maps, core_ids=list(range(N_CORES)))
    _last["exec_time_ns"] = res.exec_time_ns
    if res.instructions_and_trace is not None:
        _last["trace_path"] = res.instructions_and_trace[1]

    out = np.empty((6, E), np.float32)
    for r in range(6):
        sorted_scores = np.empty(EPC * N_CORES, np.float32)
        for c in range(N_CORES):
            s = res.results[c][f"scores{r}"]       # [128, T2]
            flat = s.T.reshape(-1)                 # slot j = t*128+p
            eos = slot_maps[r][c]
            valid = eos >= 0
            sorted_scores[c * EPC + eos[valid]] = flat[valid]
        out[r, perms[r]] = sorted_scores
    return out
